# revision 41
# baseline (speedup 1.0000x reference)
"""Causal self-attention (B=4, S=2048, D=2048, H=16) on 8 TRN2 NeuronCores.

Sharding: core c -> batch b=c//2, tensor-parallel half t=c%2 (8 heads each).
Each core computes QKV projections for its 8 heads, causal attention, and a
partial out-projection; the host sums the two TP partials per batch and
applies the 1/SW^2 weight-scale correction.

The schedule is built around the PE *sequencer* budget (~120 ns decode per
Ldweights+Matmult pair): projections run as fp8 DoubleRow triples with
1024-wide moving operands into 2-bank PSUM tiles, and q/k/v/ctx all stay
resident in SBUF (no DRAM spill round-trips).  Attention (scores, softmax,
attn@V) runs in fp16 with exp() on pairs of key tiles; the softmax
denominator is reduced+broadcast on the idle GPSIMD engine so the PE does
only the two real matmuls per key tile.

The PE instruction stream is software-pipelined: attention J-tiles for one
phase are interleaved (at matmul-triple granularity) with the next
projection phase / out-projection, so the scores->exp->attn@V dependency
latency is hidden behind independent projection work and the PE stays
engine-bound end to end.
"""
import math
from collections import deque
from contextlib import ExitStack

import ml_dtypes
import numpy as np

import concourse.bass as bass
import concourse.bass_isa as bass_isa
import concourse.bacc as bacc
import concourse.mybir as mybir
import concourse.tile as tile
from concourse.bass_utils import run_bass_kernel_spmd

B, S, D, H, HD = 4, 2048, 2048, 16, 128
HL = 8              # heads per core
ML = HL * HD        # local model dim (1024)
P = 128
NDT = D // P        # 16 contraction tiles
NDP = NDT // 2      # 8 contraction pair-tiles for DoubleRow
NST = S // P        # 16 seq tiles
SW = 16.0           # host-side weight scale into e4m3 normal range
ISQ = 1.0 / math.sqrt(HD)
ESCALE = ISQ / (SW * SW)
EBIAS = -math.log(16.0)   # exp bias: keeps fp16 row sums < 64k
OSCALE = 1.0 / (SW * SW)  # applied host-side to the summed partials
F32 = mybir.dt.float32
F16 = mybir.dt.float16
F8H = mybir.dt.float8e4
F8L = mybir.dt.float8e5
Exp = mybir.ActivationFunctionType.Exp
DRM = mybir.MatmulPerfMode.DoubleRow

_BUILT = {}


class Units:
    """FIFO of (pe_cost_ns, emit_fn, label) closures — the background PE
    stream. Labels mark producer boundaries so a consumer can force-drain
    everything it depends on before emitting its own instructions."""

    def __init__(self):
        self.q = deque()
        self.done = set()
        self.credit = 0.0

    def add(self, cost, fn, label=None):
        self.q.append((cost, fn, label))

    def _pop(self):
        c, fn, label = self.q.popleft()
        fn()
        if label is not None:
            self.done.add(label)
        return c

    def drain(self, ns):
        # credit-based: pop only when enough PE-time credit has accrued, so
        # the background stream is spread evenly instead of drying up early
        self.credit += ns
        while self.q and self.credit >= self.q[0][0]:
            self.credit -= self._pop()
        if not self.q:
            self.credit = 0.0

    def drain_until(self, label):
        while label not in self.done and self.q:
            self._pop()

    def drain_all(self):
        while self.q:
            self._pop()


def _build():
    nc = bacc.Bacc("TRN2", target_bir_lowering=False, debug=False,
                   num_devices=8)
    x4p = nc.declare_dram_parameter("x4", [P, 14, S], F8H, isOutput=False)
    x5p = nc.declare_dram_parameter("x5", [P, 14, S], F8L, isOutput=False)
    x6p = nc.declare_dram_parameter("x6", [P, 2, S], F16, isOutput=False)
    wq4p = nc.declare_dram_parameter("wq4", [P, HL, 14, P], F8H,
                                     isOutput=False)
    wq5p = nc.declare_dram_parameter("wq5", [P, HL, 14, P], F8L,
                                     isOutput=False)
    wq6p = nc.declare_dram_parameter("wq6", [P, HL, 2, P], F16,
                                     isOutput=False)
    wk4p = nc.declare_dram_parameter("wk4", [P, HL, 14, P], F8H,
                                     isOutput=False)
    wk5p = nc.declare_dram_parameter("wk5", [P, HL, 14, P], F8L,
                                     isOutput=False)
    wk6p = nc.declare_dram_parameter("wk6", [P, HL, 2, P], F16,
                                     isOutput=False)
    wv4p = nc.declare_dram_parameter("wv4", [P, 14, ML], F8H, isOutput=False)
    wv5p = nc.declare_dram_parameter("wv5", [P, 14, ML], F8L, isOutput=False)
    wv6p = nc.declare_dram_parameter("wv6", [P, 2, ML], F16, isOutput=False)
    wo4p = nc.declare_dram_parameter("wo4", [P, HL, D], F8H, isOutput=False)
    wo5p = nc.declare_dram_parameter("wo5", [P, HL, D], F8L, isOutput=False)
    mask0 = nc.declare_dram_parameter("mask0", [P, P], F16, isOutput=False)
    outp = nc.declare_dram_parameter("out", [S, D], F16, isOutput=True)

    def dr3(ps, s4, s5, m4, m5, first, last):
        """hi*hi + hi*lo + lo*hi fp8 DoubleRow accumulation into ps."""
        nc.tensor.matmul(ps, s4, m4, start=first, stop=False, perf_mode=DRM)
        nc.tensor.matmul(ps, s4, m5, start=False, stop=False, perf_mode=DRM)
        nc.tensor.matmul(ps, s5, m4, start=False, stop=last, perf_mode=DRM)

    with tile.TileContext(nc) as tc, ExitStack() as top:
        # ---- long-lived SBUF (left side) ----
        const = top.enter_context(tc.tile_pool(name="const", bufs=1,
                                               side="left"))
        res = top.enter_context(tc.tile_pool(name="res", bufs=1, side="left"))
        pt_p = top.enter_context(tc.tile_pool(name="pt", bufs=3, side="left"))
        acc_p = top.enter_context(tc.tile_pool(name="accp", bufs=2,
                                               side="left"))
        nrm_p = top.enter_context(tc.tile_pool(name="nrm", bufs=2,
                                               side="left"))
        rec_p = top.enter_context(tc.tile_pool(name="rec1", bufs=1,
                                               side="left"))
        ostg_p = top.enter_context(tc.tile_pool(name="ostg", bufs=2,
                                                side="left"))
        # ---- phase-scoped SBUF (right side) ----
        ph1 = ExitStack()
        x_p = ph1.enter_context(tc.tile_pool(name="xp", bufs=1, side="right"))
        wqk_p = ph1.enter_context(tc.tile_pool(name="wqk", bufs=3,
                                               side="right"))
        wv_p = ph1.enter_context(tc.tile_pool(name="wvp", bufs=1,
                                              side="right"))
        # ---- PSUM ----
        pp_p = top.enter_context(tc.tile_pool(name="pp", bufs=4,
                                              space="PSUM"))
        pscr_p = top.enter_context(tc.tile_pool(name="pscr", bufs=2,
                                                space="PSUM"))
        pctx_p = top.enter_context(tc.tile_pool(name="pctx", bufs=2,
                                                space="PSUM"))

        # resident tensors
        qres = res.tile([P, HL, S], F16, tag="qres")
        kres = res.tile([P, HL, S], F16, tag="kres")
        vres = res.tile([P, NST, ML], F16, tag="vres")
        ctx4 = res.tile([P, HL, S], F8H, tag="ctx4")
        ctx5 = res.tile([P, HL, S], F8L, tag="ctx5")

        m0 = const.tile([P, P], F16, tag="m0")
        nc.sync.dma_start(out=m0[:], in_=mask0[:])
        ebias = const.tile([P, 1], F32, tag="ebias")
        nc.vector.memset(ebias[:], EBIAS)

        # x resident: current seq half only [P, NDT, 1024] per plane (ring-1
        # reuse: the g=1 load write-after-read waits on the last g=0 reader)
        xh = {}

        def dma_x_half(g):
            xh[(g, 4)] = x_p.tile([P, 14, 1024], F8H, tag="x4h",
                                  name=f"x4h{g}")
            xh[(g, 5)] = x_p.tile([P, 14, 1024], F8L, tag="x5h",
                                  name=f"x5h{g}")
            xh[(g, 6)] = x_p.tile([P, 2, 1024], F16, tag="x6h",
                                  name=f"x6h{g}")
            for t in range(7):
                for pl, xp in ((4, x4p), (5, x5p)):
                    nc.sync.dma_start(
                        out=xh[(g, pl)][:, 2 * t:2 * t + 2, :],
                        in_=xp[:, 2 * t:2 * t + 2, g * 1024:(g + 1) * 1024])
            nc.sync.dma_start(
                out=xh[(g, 6)][:], in_=x6p[:, :, g * 1024:(g + 1) * 1024])

        # ---------- projection unit builders ----------
        def qk_head_units(units, g, h, w4p, w5p, w6p, dst, nm):
            """q or k projection for head h, seq half g -> dst[:,h,g*1024:].
            Contraction pairs 0..6 run as fp8 DoubleRow triples; the last
            pair (tiles 14,15) runs as two plain fp16 matmuls — same bytes,
            better SEQ/engine balance on the sequencer-bound PE."""
            cell = {}

            def dma(cell=cell):
                cell["w4"] = wqk_p.tile([P, 14, P], F8H, tag="w4",
                                        name=f"w4{nm}{g}_{h}")
                cell["w5"] = wqk_p.tile([P, 14, P], F8L, tag="w5",
                                        name=f"w5{nm}{g}_{h}")
                cell["w6"] = wqk_p.tile([P, 2, P], F16, tag="w6",
                                        name=f"w6{nm}{g}_{h}")
                nc.sync.dma_start(out=cell["w4"][:], in_=w4p[:, h, :, :])
                nc.sync.dma_start(out=cell["w5"][:], in_=w5p[:, h, :, :])
                nc.sync.dma_start(out=cell["w6"][:], in_=w6p[:, h, :, :])
            units.add(0, dma)
            for hf in range(2):
                for t in range(7):
                    for mi in range(3):
                        def mm(t=t, mi=mi, hf=hf, cell=cell):
                            if t == 0 and mi == 0:
                                cell["pq"] = pp_p.tile(
                                    [P, 512], F32, tag="pp",
                                    name=f"p{nm}{g}_{h}_{hf}")
                            sl = slice(2 * t, 2 * t + 2)
                            cs = slice(hf * 512, (hf + 1) * 512)
                            s = cell["w4" if mi < 2 else "w5"][:, sl, :]
                            m = xh[(g, 5 if mi == 1 else 4)][:, sl, cs]
                            nc.tensor.matmul(
                                cell["pq"][:], s, m,
                                start=(t == 0 and mi == 0),
                                stop=False, perf_mode=DRM)
                        units.add(128, mm)
                for i in range(2):
                    def mm16(i=i, hf=hf, cell=cell):
                        cs = slice(hf * 512, (hf + 1) * 512)
                        nc.tensor.matmul(
                            cell["pq"][:], cell["w6"][:, i, :],
                            xh[(g, 6)][:, i, cs],
                            start=False, stop=(i == 1))
                    units.add(213, mm16)

                def evac(hf=hf, cell=cell):
                    # split evacuations between Act (q) and DVE (k) so
                    # neither engine saturates during overlap windows
                    d = dst[:, h, g * 1024 + hf * 512:
                            g * 1024 + (hf + 1) * 512]
                    if nm == "q":
                        nc.scalar.copy(d, cell["pq"][:])
                    else:
                        nc.vector.tensor_copy(d, cell["pq"][:])
                units.add(0, evac)

        def v_dma_unit(units, g, mh, cell):
            def dma(cell=cell, mh=mh, g=g):
                cell["wv4"] = wv_p.tile([P, NDT, 512], F8H, tag="wv4",
                                        name=f"wv4{g}_{mh}")
                cell["wv5"] = wv_p.tile([P, NDT, 512], F8L, tag="wv5",
                                        name=f"wv5{g}_{mh}")
                nc.sync.dma_start(
                    out=cell["wv4"][:],
                    in_=wv4p[:, :, mh * 512:(mh + 1) * 512])
                nc.sync.dma_start(
                    out=cell["wv5"][:],
                    in_=wv5p[:, :, mh * 512:(mh + 1) * 512])
            units.add(0, dma)

        def v_group_units(units, g, mh, cell, sts=range(8)):
            for st in sts:
                stg = 8 * g + st
                for t in range(NDP):
                    for mi in range(3):
                        def mm(t=t, mi=mi, stg=stg, mh=mh, cell=cell, g=g):
                            if t == 0 and mi == 0:
                                cell["pv"] = pp_p.tile(
                                    [P, 512], F32, tag="pp",
                                    name=f"pv{stg}_{mh}")
                            sl = slice(2 * t, 2 * t + 2)
                            so = (stg - 8 * g) * P
                            s = xh[(g, 5 if mi == 2 else 4)][:, sl,
                                                             so:so + P]
                            m = cell["wv5" if mi == 1 else "wv4"][:, sl, :]
                            nc.tensor.matmul(
                                cell["pv"][:], s, m,
                                start=(t == 0 and mi == 0),
                                stop=(t == NDP - 1 and mi == 2),
                                perf_mode=DRM)
                        units.add(128, mm)
                units.add(0, lambda stg=stg, mh=mh, cell=cell:
                          nc.vector.tensor_copy(
                              vres[:, stg, mh * 512:(mh + 1) * 512],
                              cell["pv"][:]))

        def proj_phase_units(g, reserve=None):
            # head-0 q/k first (small weight DMAs -> earliest PE start);
            # each v m-half's weight DMA is sandwiched between qk heads so
            # its write-after-read wait on the previous half's buffer
            # overlaps projection compute instead of stalling the PE.
            # With `reserve`, the m-half-1 v groups for s-tiles 4..7 go to
            # the reserve queue (PE filler for the following c=0 pass).
            units = Units()
            cell0, cell1 = {}, {}
            qk_head_units(units, g, 0, wk4p, wk5p, wk6p, kres, "k")
            qk_head_units(units, g, 0, wq4p, wq5p, wq6p, qres, "q")
            v_dma_unit(units, g, 0, cell0)
            v_group_units(units, g, 0, cell0)
            qk_head_units(units, g, 1, wk4p, wk5p, wk6p, kres, "k")
            qk_head_units(units, g, 1, wq4p, wq5p, wq6p, qres, "q")
            v_dma_unit(units, g, 1, cell1)
            if reserve is None:
                v_group_units(units, g, 1, cell1)
            else:
                v_group_units(units, g, 1, cell1, sts=range(0, 4))
                v_group_units(reserve, g, 1, cell1, sts=range(4, 8))
            for h in range(2, HL):
                qk_head_units(units, g, h, wk4p, wk5p, wk6p, kres, "k")
                qk_head_units(units, g, h, wq4p, wq5p, wq6p, qres, "q")
            return units

        # ---------- out-projection ----------
        wo4s = [None]
        wo5s = [None]

        def oproj_units(units, st_list, use_act):
            for st in st_list:
                for q4 in range(4):
                    po = [None]
                    for hp in range(4):
                        for mi in range(3):
                            def mm(hp=hp, mi=mi, st=st, q4=q4, po=po):
                                if hp == 0 and mi == 0:
                                    po[0] = pp_p.tile([P, 512], F32,
                                                      tag="pp",
                                                      name=f"po{st}_{q4}")
                                sl = slice(2 * hp, 2 * hp + 2)
                                ss = st * P
                                qs = q4 * 512
                                s = (ctx4 if mi < 2 else ctx5)[
                                    :, sl, ss:ss + P]
                                m = (wo5s if mi == 1 else wo4s)[0][
                                    :, sl, qs:qs + 512]
                                nc.tensor.matmul(
                                    po[0][:], s, m,
                                    start=(hp == 0 and mi == 0),
                                    stop=(hp == 3 and mi == 2),
                                    perf_mode=DRM)
                            units.add(128, mm)
                    def evac(st=st, q4=q4, po=po):
                        ev = ostg_p.tile([P, 512], F16, tag="ostg",
                                         name=f"ostg{st}_{q4}")
                        if use_act and q4 % 2 == 0:
                            nc.scalar.copy(ev[:], po[0][:])
                        else:
                            nc.vector.tensor_copy(ev[:], po[0][:])
                        nc.sync.dma_start(
                            out=outp[st * P:(st + 1) * P,
                                     q4 * 512:(q4 + 1) * 512],
                            in_=ev[:])
                    units.add(0, evac)

        # ---------- attention ----------
        def attn_chunk(h, c, bg, scale=1.0, finish_prev=None):
            nj, ndiag = 4 * c + 4, 4 * c
            d1, d2, d3 = int(250 * scale), int(200 * scale), int(400 * scale)
            acc = acc_p.tile([P, 512], F16, tag="acc", name=f"acc{h}_{c}")
            pctx = pctx_p.tile([P, 512], F32, tag="pctx", name=f"px{h}_{c}")
            pend = deque()

            def emit_av(last):
                Jp, rp, ptp = pend.popleft()
                nc.tensor.matmul(
                    pctx[:, rp:512],
                    vres[:, Jp, h * P:(h + 1) * P], ptp[:, rp:512],
                    start=(Jp == 0), stop=last)

            for J in range(nj):
                r = J * P - c * 512 if J >= ndiag else 0
                pscr = pscr_p.tile([P, 512], F32, tag="pscr",
                                   name=f"ps{h}_{c}_{J}")
                nc.tensor.matmul(
                    pscr[:, r:512],
                    kres[:, h, J * P:(J + 1) * P],
                    qres[:, h, c * 512 + r:(c + 1) * 512],
                    start=True, stop=True)
                bg.drain(d1)
                pt = pt_p.tile([P, 512], F16, tag="pt",
                               name=f"pt{h}_{c}_{J}")
                nc.scalar.activation(pt[:, r:512], pscr[:, r:512], Exp,
                                     bias=ebias[:], scale=ESCALE)
                if J >= ndiag:
                    nc.vector.tensor_mul(pt[:, r:r + P], pt[:, r:r + P],
                                         m0[:])
                if J == 0:
                    if c < 2:
                        nc.scalar.copy(acc[:], pt[:])
                    else:
                        nc.vector.tensor_copy(acc[:], pt[:])
                else:
                    nc.vector.tensor_add(acc[:, r:512], acc[:, r:512],
                                         pt[:, r:512])
                pend.append((J, r, pt))
                # lag-2: attn@V for J-2 — two full J periods behind its
                # exp(), so the Act latency never stalls the PE
                if len(pend) > 2:
                    emit_av(False)
                    bg.drain(d2)
                if J == 1 and finish_prev is not None:
                    finish_prev()
                    finish_prev = None
            if finish_prev is not None:
                finish_prev()
            while pend:
                emit_av(len(pend) == 1)
            # softmax denominator: reduce+broadcast on the idle GPSIMD engine
            den = nrm_p.tile([P, 512], F32, tag="den", name=f"den{h}_{c}")
            nc.gpsimd.partition_all_reduce(
                den[:], acc[:], channels=P, reduce_op=bass_isa.ReduceOp.add)

            def finish(bg2=bg, d3=d3):
                # deferred tail of the normalization: emitted early in the
                # NEXT chunk so the DVE never head-of-line blocks on the
                # Pool all_reduce. GPSIMD cannot read PSUM, so the pctx
                # multiply stays on DVE; the SBUF-only ops go to Pool.
                rec = rec_p.tile([P, 512], F32, tag="rec",
                                 name=f"rec{h}_{c}")
                nc.vector.reciprocal(rec[:], den[:])
                bg2.drain(d3)
                cv = nrm_p.tile([P, 512], F16, tag="cv", name=f"cv{h}_{c}")
                nc.vector.tensor_mul(cv[:], pctx[:], rec[:])
                cs = slice(c * 512, (c + 1) * 512)
                nc.gpsimd.tensor_copy(ctx4[:, h, cs], cv[:])
                nc.gpsimd.tensor_sub(ctx5[:, h, cs], cv[:], ctx4[:, h, cs])
                bg2.drain(d3)
            return finish

        # ================= schedule =================
        # Scarce background supply is rate-matched to the Act-bound
        # attention rate (~144 ns of filler per key tile): PE stays filled
        # exactly while exp() limits, leftovers run as pure PE stretches.
        RM = 0.40
        empty = Units()
        g0t = Units()
        g0 = proj_phase_units(0, reserve=g0t)
        g0.drain(1)     # head-0 weight DMAs issue before the x bulk load
        dma_x_half(0)
        g0.drain_all()

        g1 = proj_phase_units(1)
        fin = None
        for h in range(HL):
            # c=0 chunks are short (~3us): spread the whole v reserve across
            # them
            fin = attn_chunk(h, 0, g0t, scale=0.49, finish_prev=fin)
        g0t.drain_all()
        # x half-1 reload: emitted only after every half-0 reader (incl. the
        # reserved v groups) — its ~12us stream hides behind the first c=1
        # chunks, which therefore run without g1 drains
        dma_x_half(1)
        for h in range(HL):
            fin = attn_chunk(h, 1, g1 if h >= 2 else empty, scale=RM,
                             finish_prev=fin)
        fin()
        fin = None
        g1.drain_all()
        ph1.close()

        # phase 3 SBUF (reuses the space freed by x/weight pools)
        ph3 = ExitStack()
        wo_p = ph3.enter_context(tc.tile_pool(name="wop", bufs=1,
                                              side="right"))
        wo4s[0] = wo_p.tile([P, HL, D], F8H, tag="wo4s", name="wo4s")
        wo5s[0] = wo_p.tile([P, HL, D], F8L, tag="wo5s", name="wo5s")
        for half in range(2):
            hs = slice(half * 1024, (half + 1) * 1024)
            nc.sync.dma_start(out=wo4s[0][:, :, hs], in_=wo4p[:, :, hs])
            nc.sync.dma_start(out=wo5s[0][:, :, hs], in_=wo5p[:, :, hs])

        b2 = Units()
        oproj_units(b2, range(0, 8), use_act=True)
        for h in range(HL):
            # h<2: no drains, so the ~12us wo weight DMA completes behind
            # attention instead of stalling the first out-proj triple
            fin = attn_chunk(h, 2, b2 if h >= 1 else empty, scale=RM,
                             finish_prev=fin)
        oproj_units(b2, range(8, 12), use_act=True)
        for h in range(HL):
            fin = attn_chunk(h, 3, b2, scale=0.7, finish_prev=fin)
        fin()
        oproj_units(b2, range(12, 16), use_act=False)
        b2.drain_all()
        ph3.close()

    nc.finalize()
    return nc


def get_nc():
    if "nc" not in _BUILT:
        _BUILT["nc"] = _build()
    return _BUILT["nc"]


def _f8split(a):
    """Split fp32 into an e4m3 hi plane and an e5m2 lo residual plane."""
    a = np.ascontiguousarray(a, dtype=np.float32)
    hi = a.astype(ml_dtypes.float8_e4m3)
    lo = (a - hi.astype(np.float32)).astype(ml_dtypes.float8_e5m2)
    return hi, lo


def _make_in_maps(x, Wq, Wk, Wv, Wo):
    jj, ff = np.meshgrid(np.arange(P), np.arange(P), indexing="ij")
    m0 = (ff >= jj).astype(np.float16)
    in_maps = []
    for c in range(8):
        b, t = c // 2, c % 2
        ms = slice(t * ML, (t + 1) * ML)
        # x^T [D,S] -> [P, NDT, S]
        x4, x5 = _f8split(
            x[b].T.reshape(NDT, P, S).transpose(1, 0, 2))
        # Wq/Wk slices: (SW*W[ms,:]).T [D, ML] -> [P, HL, NDT, P]
        wq4, wq5 = _f8split(
            (SW * Wq[ms, :]).T.reshape(NDT, P, HL, P).transpose(1, 2, 0, 3))
        wk4, wk5 = _f8split(
            (SW * Wk[ms, :]).T.reshape(NDT, P, HL, P).transpose(1, 2, 0, 3))
        # Wv: (SW*Wv[ms,:]).T [D, ML] -> [P, NDT, ML]
        wv4, wv5 = _f8split(
            (SW * Wv[ms, :]).T.reshape(NDT, P, ML).transpose(1, 0, 2))
        # Wo: (SW*Wo[:,ms]).T [ML, D] -> [P, HL, D]
        wo4, wo5 = _f8split(
            (SW * Wo[:, ms]).T.reshape(HL, P, D).transpose(1, 0, 2))
        in_maps.append({
            "x4": x4, "x5": x5,
            "wq4": wq4, "wq5": wq5,
            "wk4": wk4, "wk5": wk5,
            "wv4": wv4, "wv5": wv5,
            "wo4": wo4, "wo5": wo5,
            "mask0": m0,
        })
    return in_maps


def kernel(x, Wq, Wk, Wv, Wo):
    x = np.asarray(x, dtype=np.float32)
    Wq = np.asarray(Wq, dtype=np.float32)
    Wk = np.asarray(Wk, dtype=np.float32)
    Wv = np.asarray(Wv, dtype=np.float32)
    Wo = np.asarray(Wo, dtype=np.float32)

    nc = get_nc()
    in_maps = _make_in_maps(x, Wq, Wk, Wv, Wo)
    res = run_bass_kernel_spmd(nc, in_maps, list(range(8)))
    outs = [res.results[c]["out"].astype(np.float32) for c in range(8)]
    full = np.stack([(outs[2 * b] + outs[2 * b + 1]) * OSCALE
                     for b in range(B)])
    return full.astype(np.float32)


# revision 58
# speedup vs baseline: 1.0180x; 1.0180x over previous
"""Causal self-attention (B=4, S=2048, D=2048, H=16) on 8 TRN2 NeuronCores.

Sharding: core c -> batch b=c//2, tensor-parallel half t=c%2 (8 heads each).
Each core computes QKV projections for its 8 heads, causal attention, and a
partial out-projection; the host sums the two TP partials per batch and
applies the 1/SW^2 weight-scale correction.

Projections and out-projection run as fp8 DoubleRow triples (e4m3 hi plane +
e5m2 residual plane, weights pre-scaled by SW=16) with 512-wide moving
operands — one PSUM bank per accumulation group. q/k/v and the fp8 context
planes stay resident in SBUF, so attention reads them directly and the only
DRAM traffic is x, the streamed weights, and the fp16 output partials.

Attention (scores, softmax, attn@V) runs in fp16. The softmax denominator is
a GPSIMD partition_all_reduce (sum+broadcast in one op on the otherwise-idle
engine); the normalization tail of each chunk is emitted early in the NEXT
chunk so the DVE never head-of-line blocks on the Pool engine.

The schedule is built around the PE sequencer budget (~120 ns decode per
Ldweights+Matmult pair, which makes the PE co-bound between sequencer and
engine). The PE stream is software-pipelined: attention key-tiles are
interleaved, at single-matmul granularity and with a credit-based
rate-matched drain, with the next projection phase (pass 1) or the
out-projection (pass 2); attn@V lags its scores by two key tiles so the
exp() latency on the Act engine never stalls the PE.
"""
import math
from collections import deque
from contextlib import ExitStack

import ml_dtypes
import numpy as np

import concourse.bass as bass
import concourse.bass_isa as bass_isa
import concourse.bacc as bacc
import concourse.mybir as mybir
import concourse.tile as tile
from concourse.bass_utils import run_bass_kernel_spmd

B, S, D, H, HD = 4, 2048, 2048, 16, 128
HL = 8              # heads per core
ML = HL * HD        # local model dim (1024)
P = 128
NDT = D // P        # 16 contraction tiles
NDP = NDT // 2      # 8 contraction pair-tiles for DoubleRow
NST = S // P        # 16 seq tiles
SW = 16.0           # host-side weight scale into e4m3 normal range
ISQ = 1.0 / math.sqrt(HD)
ESCALE = ISQ / (SW * SW)
EBIAS = -math.log(16.0)   # exp bias: keeps fp16 row sums < 64k
OSCALE = 1.0 / (SW * SW)  # applied host-side to the summed partials
F32 = mybir.dt.float32
F16 = mybir.dt.float16
F8H = mybir.dt.float8e4
F8L = mybir.dt.float8e5
Exp = mybir.ActivationFunctionType.Exp
DRM = mybir.MatmulPerfMode.DoubleRow

_BUILT = {}


class Units:
    """FIFO of (pe_cost_ns, emit_fn) closures — the background PE stream."""

    def __init__(self):
        self.q = deque()
        self.credit = 0.0

    def add(self, cost, fn):
        self.q.append((cost, fn))

    def _pop(self):
        c, fn = self.q.popleft()
        fn()
        return c

    def drain(self, ns):
        # credit-based: pop only when enough PE-time credit has accrued, so
        # the background stream is spread evenly instead of drying up early
        self.credit += ns
        while self.q and self.credit >= self.q[0][0]:
            self.credit -= self._pop()
        if not self.q:
            self.credit = 0.0

    def drain_all(self):
        while self.q:
            self._pop()


def _build():
    nc = bacc.Bacc("TRN2", target_bir_lowering=False, debug=False,
                   num_devices=8)
    x4p = nc.declare_dram_parameter("x4", [P, NDT, S], F8H, isOutput=False)
    x5p = nc.declare_dram_parameter("x5", [P, NDT, S], F8L, isOutput=False)
    wq4p = nc.declare_dram_parameter("wq4", [P, HL, NDT, P], F8H,
                                     isOutput=False)
    wq5p = nc.declare_dram_parameter("wq5", [P, HL, NDT, P], F8L,
                                     isOutput=False)
    wk4p = nc.declare_dram_parameter("wk4", [P, HL, NDT, P], F8H,
                                     isOutput=False)
    wk5p = nc.declare_dram_parameter("wk5", [P, HL, NDT, P], F8L,
                                     isOutput=False)
    wv4p = nc.declare_dram_parameter("wv4", [P, NDT, ML], F8H, isOutput=False)
    wv5p = nc.declare_dram_parameter("wv5", [P, NDT, ML], F8L, isOutput=False)
    wo4p = nc.declare_dram_parameter("wo4", [P, HL, D], F8H, isOutput=False)
    wo5p = nc.declare_dram_parameter("wo5", [P, HL, D], F8L, isOutput=False)
    mask0 = nc.declare_dram_parameter("mask0", [P, P], F16, isOutput=False)
    outp = nc.declare_dram_parameter("out", [S, D], F16, isOutput=True)

    with tile.TileContext(nc) as tc, ExitStack() as top:
        # ---- long-lived SBUF (left side) ----
        const = top.enter_context(tc.tile_pool(name="const", bufs=1,
                                               side="left"))
        res = top.enter_context(tc.tile_pool(name="res", bufs=1, side="left"))
        pt_p = top.enter_context(tc.tile_pool(name="pt", bufs=4, side="left"))
        acc_p = top.enter_context(tc.tile_pool(name="accp", bufs=2,
                                               side="left"))
        nrm_p = top.enter_context(tc.tile_pool(name="nrm", bufs=2,
                                               side="left"))
        rec_p = top.enter_context(tc.tile_pool(name="rec1", bufs=1,
                                               side="left"))
        ostg_p = top.enter_context(tc.tile_pool(name="ostg", bufs=2,
                                                side="left"))
        # ---- phase-scoped SBUF (right side) ----
        ph1 = ExitStack()
        x_p = ph1.enter_context(tc.tile_pool(name="xp", bufs=1, side="right"))
        wqk_p = ph1.enter_context(tc.tile_pool(name="wqk", bufs=3,
                                               side="right"))
        wv_p = ph1.enter_context(tc.tile_pool(name="wvp", bufs=1,
                                              side="right"))
        # ---- PSUM ----
        pp_p = top.enter_context(tc.tile_pool(name="pp", bufs=4,
                                              space="PSUM"))
        pscr_p = top.enter_context(tc.tile_pool(name="pscr", bufs=2,
                                                space="PSUM"))
        pctx_p = top.enter_context(tc.tile_pool(name="pctx", bufs=2,
                                                space="PSUM"))

        # resident tensors
        qres = res.tile([P, HL, S], F16, tag="qres")
        kres = res.tile([P, HL, S], F16, tag="kres")
        vres = res.tile([P, NST, ML], F16, tag="vres")
        ctx4 = res.tile([P, HL, S], F8H, tag="ctx4")
        ctx5 = res.tile([P, HL, S], F8L, tag="ctx5")

        m0 = const.tile([P, P], F16, tag="m0")
        ebias = const.tile([P, 1], F32, tag="ebias")
        nc.vector.memset(ebias[:], EBIAS)

        # x resident: current seq half only [P, NDT, 1024] per plane (ring-1
        # reuse: the g=1 load write-after-read waits on the last g=0 reader)
        xh = {}

        def dma_x_half(g):
            xh[(g, 4)] = x_p.tile([P, NDT, 1024], F8H, tag="x4h",
                                  name=f"x4h{g}")
            xh[(g, 5)] = x_p.tile([P, NDT, 1024], F8L, tag="x5h",
                                  name=f"x5h{g}")
            for t in range(NDP):
                for pl, xp in ((4, x4p), (5, x5p)):
                    nc.sync.dma_start(
                        out=xh[(g, pl)][:, 2 * t:2 * t + 2, :],
                        in_=xp[:, 2 * t:2 * t + 2, g * 1024:(g + 1) * 1024])

        # ---------- projection unit builders ----------
        def qk_head_units(units, g, h, w4p, w5p, dst, nm):
            """q or k projection for head h, seq half g -> dst[:,h,g*1024:]."""
            cell = {}

            def dma(cell=cell):
                cell["w4"] = wqk_p.tile([P, NDT, P], F8H, tag="w4",
                                        name=f"w4{nm}{g}_{h}")
                cell["w5"] = wqk_p.tile([P, NDT, P], F8L, tag="w5",
                                        name=f"w5{nm}{g}_{h}")
                nc.sync.dma_start(out=cell["w4"][:], in_=w4p[:, h, :, :])
                nc.sync.dma_start(out=cell["w5"][:], in_=w5p[:, h, :, :])
            units.add(0, dma)
            for hf in range(2):
                for t in range(NDP):
                    for mi in range(3):
                        def mm(t=t, mi=mi, hf=hf, cell=cell):
                            if t == 0 and mi == 0:
                                cell["pq"] = pp_p.tile(
                                    [P, 512], F32, tag="pp",
                                    name=f"p{nm}{g}_{h}_{hf}")
                            sl = slice(2 * t, 2 * t + 2)
                            cs = slice(hf * 512, (hf + 1) * 512)
                            s = cell["w4" if mi < 2 else "w5"][:, sl, :]
                            m = xh[(g, 5 if mi == 1 else 4)][:, sl, cs]
                            nc.tensor.matmul(
                                cell["pq"][:], s, m,
                                start=(t == 0 and mi == 0),
                                stop=(t == NDP - 1 and mi == 2),
                                perf_mode=DRM)
                        units.add(128, mm)

                def evac(hf=hf, cell=cell):
                    # split evacuations between Act (q) and DVE (k) so
                    # neither engine saturates during overlap windows
                    d = dst[:, h, g * 1024 + hf * 512:
                            g * 1024 + (hf + 1) * 512]
                    if nm == "q":
                        nc.scalar.copy(d, cell["pq"][:])
                    else:
                        nc.vector.tensor_copy(d, cell["pq"][:])
                units.add(0, evac)

        def v_dma_unit(units, g, mh, cell):
            def dma(cell=cell, mh=mh, g=g):
                cell["wv4"] = wv_p.tile([P, NDT, 512], F8H, tag="wv4",
                                        name=f"wv4{g}_{mh}")
                cell["wv5"] = wv_p.tile([P, NDT, 512], F8L, tag="wv5",
                                        name=f"wv5{g}_{mh}")
                nc.sync.dma_start(
                    out=cell["wv4"][:],
                    in_=wv4p[:, :, mh * 512:(mh + 1) * 512])
                nc.sync.dma_start(
                    out=cell["wv5"][:],
                    in_=wv5p[:, :, mh * 512:(mh + 1) * 512])
            units.add(0, dma)

        def v_group_units(units, g, mh, cell, sts=range(8)):
            for st in sts:
                stg = 8 * g + st
                for t in range(NDP):
                    for mi in range(3):
                        def mm(t=t, mi=mi, stg=stg, mh=mh, cell=cell, g=g):
                            if t == 0 and mi == 0:
                                cell["pv"] = pp_p.tile(
                                    [P, 512], F32, tag="pp",
                                    name=f"pv{stg}_{mh}")
                            sl = slice(2 * t, 2 * t + 2)
                            so = (stg - 8 * g) * P
                            s = xh[(g, 5 if mi == 2 else 4)][:, sl,
                                                             so:so + P]
                            m = cell["wv5" if mi == 1 else "wv4"][:, sl, :]
                            nc.tensor.matmul(
                                cell["pv"][:], s, m,
                                start=(t == 0 and mi == 0),
                                stop=(t == NDP - 1 and mi == 2),
                                perf_mode=DRM)
                        units.add(128, mm)
                units.add(0, lambda stg=stg, mh=mh, cell=cell:
                          nc.vector.tensor_copy(
                              vres[:, stg, mh * 512:(mh + 1) * 512],
                              cell["pv"][:]))

        def proj_phase_units(g, reserve=None):
            # head-0 q/k first (small weight DMAs -> earliest PE start);
            # each v m-half's weight DMA is sandwiched between qk heads so
            # its write-after-read wait on the previous half's buffer
            # overlaps projection compute instead of stalling the PE.
            # With `reserve`, the m-half-1 v groups for s-tiles 4..7 go to
            # the reserve queue (PE filler for the following c=0 pass).
            units = Units()
            cell0, cell1 = {}, {}
            qk_head_units(units, g, 0, wk4p, wk5p, kres, "k")
            qk_head_units(units, g, 0, wq4p, wq5p, qres, "q")
            v_dma_unit(units, g, 0, cell0)
            v_group_units(units, g, 0, cell0)
            qk_head_units(units, g, 1, wk4p, wk5p, kres, "k")
            qk_head_units(units, g, 1, wq4p, wq5p, qres, "q")
            v_dma_unit(units, g, 1, cell1)
            if reserve is None:
                v_group_units(units, g, 1, cell1)
            else:
                v_group_units(units, g, 1, cell1, sts=range(0, 4))
                v_group_units(reserve, g, 1, cell1, sts=range(4, 8))
            for h in range(2, HL):
                qk_head_units(units, g, h, wk4p, wk5p, kres, "k")
                qk_head_units(units, g, h, wq4p, wq5p, qres, "q")
            return units

        # ---------- out-projection ----------
        wo4s = [None]
        wo5s = [None]

        def oproj_units(units, st_list, use_act):
            for st in st_list:
                for q4 in range(4):
                    po = [None]
                    for hp in range(4):
                        for mi in range(3):
                            def mm(hp=hp, mi=mi, st=st, q4=q4, po=po):
                                if hp == 0 and mi == 0:
                                    po[0] = pp_p.tile([P, 512], F32,
                                                      tag="pp",
                                                      name=f"po{st}_{q4}")
                                sl = slice(2 * hp, 2 * hp + 2)
                                ss = st * P
                                qs = q4 * 512
                                s = (ctx4 if mi < 2 else ctx5)[
                                    :, sl, ss:ss + P]
                                m = (wo5s if mi == 1 else wo4s)[0][
                                    :, sl, qs:qs + 512]
                                nc.tensor.matmul(
                                    po[0][:], s, m,
                                    start=(hp == 0 and mi == 0),
                                    stop=(hp == 3 and mi == 2),
                                    perf_mode=DRM)
                            units.add(128, mm)
                    def evac(st=st, q4=q4, po=po):
                        ev = ostg_p.tile([P, 512], F16, tag="ostg",
                                         name=f"ostg{st}_{q4}")
                        if use_act and q4 % 2 == 0:
                            nc.scalar.copy(ev[:], po[0][:])
                        else:
                            nc.vector.tensor_copy(ev[:], po[0][:])
                        nc.sync.dma_start(
                            out=outp[st * P:(st + 1) * P,
                                     q4 * 512:(q4 + 1) * 512],
                            in_=ev[:])
                    units.add(0, evac)

        # ---------- attention ----------
        def attn_chunk(h, c, bg, scale=1.0, finish_prev=None):
            nj, ndiag = 4 * c + 4, 4 * c
            d1, d2, d3 = int(250 * scale), int(200 * scale), int(400 * scale)
            acc = acc_p.tile([P, 512], F16, tag="acc", name=f"acc{h}_{c}")
            pctx = pctx_p.tile([P, 512], F32, tag="pctx", name=f"px{h}_{c}")
            pend = deque()

            def emit_av(last):
                Jp, rp, ptp = pend.popleft()
                nc.tensor.matmul(
                    pctx[:, rp:512],
                    vres[:, Jp, h * P:(h + 1) * P], ptp[:, rp:512],
                    start=(Jp == 0), stop=last)

            for J in range(nj):
                r = J * P - c * 512 if J >= ndiag else 0
                pscr = pscr_p.tile([P, 512], F32, tag="pscr",
                                   name=f"ps{h}_{c}_{J}")
                nc.tensor.matmul(
                    pscr[:, r:512],
                    kres[:, h, J * P:(J + 1) * P],
                    qres[:, h, c * 512 + r:(c + 1) * 512],
                    start=True, stop=True)
                bg.drain(d1)
                pt = pt_p.tile([P, 512], F16, tag="pt",
                               name=f"pt{h}_{c}_{J}")
                nc.scalar.activation(pt[:, r:512], pscr[:, r:512], Exp,
                                     bias=ebias[:], scale=ESCALE)
                if J >= ndiag:
                    nc.vector.tensor_mul(pt[:, r:r + P], pt[:, r:r + P],
                                         m0[:])
                if J == 0:
                    if c < 2:
                        nc.scalar.copy(acc[:], pt[:])
                    else:
                        nc.vector.tensor_copy(acc[:], pt[:])
                else:
                    nc.vector.tensor_add(acc[:, r:512], acc[:, r:512],
                                         pt[:, r:512])
                pend.append((J, r, pt))
                # lag-2: attn@V for J-2 — two full J periods behind its
                # exp(), so the Act latency never stalls the PE
                if len(pend) > 2:
                    emit_av(False)
                    bg.drain(d2)
                if J == 1 and finish_prev is not None:
                    finish_prev()
                    finish_prev = None
            if finish_prev is not None:
                finish_prev()
            while pend:
                emit_av(len(pend) == 1)
            # softmax denominator: reduce+broadcast on the idle GPSIMD engine
            den = nrm_p.tile([P, 512], F32, tag="den", name=f"den{h}_{c}")
            nc.gpsimd.partition_all_reduce(
                den[:], acc[:], channels=P, reduce_op=bass_isa.ReduceOp.add)

            def finish(bg2=bg, d3=d3, fast=(c == 3 and h == HL - 1)):
                # deferred tail of the normalization: emitted early in the
                # NEXT chunk so the DVE never head-of-line blocks on the
                # Pool all_reduce. GPSIMD cannot read PSUM, so the pctx
                # multiply stays on DVE; the SBUF-only ops go to Pool.
                rec = rec_p.tile([P, 512], F32, tag="rec",
                                 name=f"rec{h}_{c}")
                nc.vector.reciprocal(rec[:], den[:])
                bg2.drain(d3)
                cv = nrm_p.tile([P, 512], F16, tag="cv", name=f"cv{h}_{c}")
                nc.vector.tensor_mul(cv[:], pctx[:], rec[:])
                cs = slice(c * 512, (c + 1) * 512)
                if fast:
                    nc.scalar.copy(ctx4[:, h, cs], cv[:])
                    nc.vector.tensor_sub(ctx5[:, h, cs], cv[:],
                                         ctx4[:, h, cs])
                else:
                    nc.gpsimd.tensor_copy(ctx4[:, h, cs], cv[:])
                    nc.gpsimd.tensor_sub(ctx5[:, h, cs], cv[:],
                                         ctx4[:, h, cs])
                bg2.drain(d3)
            return finish

        # ================= schedule =================
        # Scarce background supply is rate-matched to the Act-bound
        # attention rate (~144 ns of filler per key tile): PE stays filled
        # exactly while exp() limits, leftovers run as pure PE stretches.
        RM = 0.75
        empty = Units()
        g0t = Units()
        g0 = proj_phase_units(0, reserve=g0t)
        g0.drain(1)     # head-0 weight DMAs issue before the x bulk load
        dma_x_half(0)
        nc.sync.dma_start(out=m0[:], in_=mask0[:])
        g0.drain_all()

        g1 = proj_phase_units(1)
        fin = None
        for h in range(HL):
            # c=0 chunks are short (~3us): spread the whole v reserve across
            # them
            fin = attn_chunk(h, 0, g0t, scale=0.65, finish_prev=fin)
        g0t.drain_all()
        # x half-1 reload: emitted only after every half-0 reader (incl. the
        # reserved v groups) — its ~12us stream hides behind the first c=1
        # chunks, which therefore run without g1 drains
        dma_x_half(1)
        for h in range(HL):
            fin = attn_chunk(h, 1, g1 if h >= 2 else empty, scale=RM,
                             finish_prev=fin)
        fin()
        fin = None
        g1.drain_all()
        ph1.close()

        # phase 3 SBUF (reuses the space freed by x/weight pools)
        ph3 = ExitStack()
        wo_p = ph3.enter_context(tc.tile_pool(name="wop", bufs=1,
                                              side="right"))
        wo4s[0] = wo_p.tile([P, HL, D], F8H, tag="wo4s", name="wo4s")
        wo5s[0] = wo_p.tile([P, HL, D], F8L, tag="wo5s", name="wo5s")
        for half in range(2):
            hs = slice(half * 1024, (half + 1) * 1024)
            nc.sync.dma_start(out=wo4s[0][:, :, hs], in_=wo4p[:, :, hs])
            nc.sync.dma_start(out=wo5s[0][:, :, hs], in_=wo5p[:, :, hs])

        b2 = Units()
        oproj_units(b2, range(0, 8), use_act=True)
        for h in range(HL):
            # h<2: no drains, so the ~12us wo weight DMA completes behind
            # attention instead of stalling the first out-proj triple
            fin = attn_chunk(h, 2, b2 if h >= 1 else empty, scale=RM,
                             finish_prev=fin)
        oproj_units(b2, range(8, 12), use_act=True)
        for h in range(HL):
            fin = attn_chunk(h, 3, b2, scale=0.7, finish_prev=fin)
        fin()
        oproj_units(b2, range(12, 16), use_act=False)
        b2.drain_all()
        ph3.close()

    nc.finalize()
    return nc


def get_nc():
    if "nc" not in _BUILT:
        _BUILT["nc"] = _build()
    return _BUILT["nc"]


def _f8split(a):
    """Split fp32 into an e4m3 hi plane and an e5m2 lo residual plane."""
    a = np.ascontiguousarray(a, dtype=np.float32)
    hi = a.astype(ml_dtypes.float8_e4m3)
    lo = (a - hi.astype(np.float32)).astype(ml_dtypes.float8_e5m2)
    return hi, lo


def _make_in_maps(x, Wq, Wk, Wv, Wo):
    jj, ff = np.meshgrid(np.arange(P), np.arange(P), indexing="ij")
    m0 = (ff >= jj).astype(np.float16)
    in_maps = []
    for c in range(8):
        b, t = c // 2, c % 2
        ms = slice(t * ML, (t + 1) * ML)
        # x^T [D,S] -> [P, NDT, S]
        x4, x5 = _f8split(
            x[b].T.reshape(NDT, P, S).transpose(1, 0, 2))
        # Wq/Wk slices: (SW*W[ms,:]).T [D, ML] -> [P, HL, NDT, P]
        wq4, wq5 = _f8split(
            (SW * Wq[ms, :]).T.reshape(NDT, P, HL, P).transpose(1, 2, 0, 3))
        wk4, wk5 = _f8split(
            (SW * Wk[ms, :]).T.reshape(NDT, P, HL, P).transpose(1, 2, 0, 3))
        # Wv: (SW*Wv[ms,:]).T [D, ML] -> [P, NDT, ML]
        wv4, wv5 = _f8split(
            (SW * Wv[ms, :]).T.reshape(NDT, P, ML).transpose(1, 0, 2))
        # Wo: (SW*Wo[:,ms]).T [ML, D] -> [P, HL, D]
        wo4, wo5 = _f8split(
            (SW * Wo[:, ms]).T.reshape(HL, P, D).transpose(1, 0, 2))
        in_maps.append({
            "x4": x4, "x5": x5,
            "wq4": wq4, "wq5": wq5,
            "wk4": wk4, "wk5": wk5,
            "wv4": wv4, "wv5": wv5,
            "wo4": wo4, "wo5": wo5,
            "mask0": m0,
        })
    return in_maps


def kernel(x, Wq, Wk, Wv, Wo):
    x = np.asarray(x, dtype=np.float32)
    Wq = np.asarray(Wq, dtype=np.float32)
    Wk = np.asarray(Wk, dtype=np.float32)
    Wv = np.asarray(Wv, dtype=np.float32)
    Wo = np.asarray(Wo, dtype=np.float32)

    nc = get_nc()
    in_maps = _make_in_maps(x, Wq, Wk, Wv, Wo)
    res = run_bass_kernel_spmd(nc, in_maps, list(range(8)))
    outs = [res.results[c]["out"].astype(np.float32) for c in range(8)]
    full = np.stack([(outs[2 * b] + outs[2 * b + 1]) * OSCALE
                     for b in range(B)])
    return full.astype(np.float32)


# revision 65
# speedup vs baseline: 1.0223x; 1.0042x over previous
"""Causal self-attention (B=4, S=2048, D=2048, H=16) on 8 TRN2 NeuronCores.

Sharding: core c -> batch b=c//2, tensor-parallel half t=c%2 (8 heads each).
Each core computes QKV projections for its 8 heads, causal attention, and a
partial out-projection; the host sums the two TP partials per batch and
applies the 1/SW^2 weight-scale correction.

Projections and out-projection run as fp8 DoubleRow triples (e4m3 hi plane +
e5m2 residual plane, weights pre-scaled by SW=16) with 512-wide moving
operands — one PSUM bank per accumulation group. q/k/v and the fp8 context
planes stay resident in SBUF, so attention reads them directly and the only
DRAM traffic is x, the streamed weights, and the fp16 output partials.

Attention (scores, softmax, attn@V) runs in fp16. The softmax denominator is
a GPSIMD partition_all_reduce (sum+broadcast in one op on the otherwise-idle
engine); the normalization tail of each chunk is emitted early in the NEXT
chunk so the DVE never head-of-line blocks on the Pool engine.

The schedule is built around the PE sequencer budget (~120 ns decode per
Ldweights+Matmult pair, which makes the PE co-bound between sequencer and
engine). The PE stream is software-pipelined: attention key-tiles are
interleaved, at single-matmul granularity and with a credit-based
rate-matched drain, with the next projection phase (pass 1) or the
out-projection (pass 2); attn@V lags its scores by two key tiles so the
exp() latency on the Act engine never stalls the PE.
"""
import math
from collections import deque
from contextlib import ExitStack

import ml_dtypes
import numpy as np

import concourse.bass as bass
import concourse.bass_isa as bass_isa
import concourse.bacc as bacc
import concourse.mybir as mybir
import concourse.tile as tile
from concourse.bass_utils import run_bass_kernel_spmd

B, S, D, H, HD = 4, 2048, 2048, 16, 128
HL = 8              # heads per core
ML = HL * HD        # local model dim (1024)
P = 128
NDT = D // P        # 16 contraction tiles
NDP = NDT // 2      # 8 contraction pair-tiles for DoubleRow
NST = S // P        # 16 seq tiles
SW = 16.0           # host-side weight scale into e4m3 normal range
ISQ = 1.0 / math.sqrt(HD)
ESCALE = ISQ / (SW * SW)
EBIAS = -math.log(16.0)   # exp bias: keeps fp16 row sums < 64k
OSCALE = 1.0 / (SW * SW)  # applied host-side to the summed partials
F32 = mybir.dt.float32
F16 = mybir.dt.float16
F8H = mybir.dt.float8e4
F8L = mybir.dt.float8e5
Exp = mybir.ActivationFunctionType.Exp
DRM = mybir.MatmulPerfMode.DoubleRow

_BUILT = {}


class Units:
    """FIFO of (pe_cost_ns, emit_fn) closures — the background PE stream."""

    def __init__(self):
        self.q = deque()
        self.credit = 0.0

    def add(self, cost, fn):
        self.q.append((cost, fn))

    def _pop(self):
        c, fn = self.q.popleft()
        fn()
        return c

    def drain(self, ns):
        # credit-based: pop only when enough PE-time credit has accrued, so
        # the background stream is spread evenly instead of drying up early
        self.credit += ns
        while self.q and self.credit >= self.q[0][0]:
            self.credit -= self._pop()
        if not self.q:
            self.credit = 0.0

    def drain_all(self):
        while self.q:
            self._pop()


def _build():
    nc = bacc.Bacc("TRN2", target_bir_lowering=False, debug=False,
                   num_devices=8)
    x4p = nc.declare_dram_parameter("x4", [P, NDT, S], F8H, isOutput=False)
    x5p = nc.declare_dram_parameter("x5", [P, NDT, S], F8L, isOutput=False)
    wq4p = nc.declare_dram_parameter("wq4", [P, HL, NDT, P], F8H,
                                     isOutput=False)
    wq5p = nc.declare_dram_parameter("wq5", [P, HL, NDT, P], F8L,
                                     isOutput=False)
    wk4p = nc.declare_dram_parameter("wk4", [P, HL, NDT, P], F8H,
                                     isOutput=False)
    wk5p = nc.declare_dram_parameter("wk5", [P, HL, NDT, P], F8L,
                                     isOutput=False)
    wv4p = nc.declare_dram_parameter("wv4", [P, NDT, ML], F8H, isOutput=False)
    wv5p = nc.declare_dram_parameter("wv5", [P, NDT, ML], F8L, isOutput=False)
    wo4p = nc.declare_dram_parameter("wo4", [P, HL, D], F8H, isOutput=False)
    wo5p = nc.declare_dram_parameter("wo5", [P, HL, D], F8L, isOutput=False)
    mask0 = nc.declare_dram_parameter("mask0", [P, P], F16, isOutput=False)
    outp = nc.declare_dram_parameter("out", [S, D], F16, isOutput=True)

    with tile.TileContext(nc) as tc, ExitStack() as top:
        # ---- long-lived SBUF (left side) ----
        const = top.enter_context(tc.tile_pool(name="const", bufs=1,
                                               side="left"))
        res = top.enter_context(tc.tile_pool(name="res", bufs=1, side="left"))
        pt_p = top.enter_context(tc.tile_pool(name="pt", bufs=4, side="left"))
        acc_p = top.enter_context(tc.tile_pool(name="accp", bufs=2,
                                               side="left"))
        nrm_p = top.enter_context(tc.tile_pool(name="nrm", bufs=2,
                                               side="left"))
        rec_p = top.enter_context(tc.tile_pool(name="rec1", bufs=1,
                                               side="left"))
        ostg_p = top.enter_context(tc.tile_pool(name="ostg", bufs=2,
                                                side="left"))
        # ---- phase-scoped SBUF (right side) ----
        ph1 = ExitStack()
        x_p = ph1.enter_context(tc.tile_pool(name="xp", bufs=1, side="right"))
        wqk_p = ph1.enter_context(tc.tile_pool(name="wqk", bufs=3,
                                               side="right"))
        wv_p = ph1.enter_context(tc.tile_pool(name="wvp", bufs=1,
                                              side="right"))
        # ---- PSUM ----
        pp_p = top.enter_context(tc.tile_pool(name="pp", bufs=4,
                                              space="PSUM"))
        pscr_p = top.enter_context(tc.tile_pool(name="pscr", bufs=2,
                                                space="PSUM"))
        pctx_p = top.enter_context(tc.tile_pool(name="pctx", bufs=2,
                                                space="PSUM"))

        # resident tensors
        qres = res.tile([P, HL, S], F16, tag="qres")
        kres = res.tile([P, HL, S], F16, tag="kres")
        vres = res.tile([P, NST, ML], F16, tag="vres")
        ctx4 = res.tile([P, HL, S], F8H, tag="ctx4")
        ctx5 = res.tile([P, HL, S], F8L, tag="ctx5")

        m0 = const.tile([P, P], F16, tag="m0")
        ebias = const.tile([P, 1], F32, tag="ebias")
        nc.vector.memset(ebias[:], EBIAS)

        # x resident: current seq half only, as two column-half tiles per
        # plane (ring-1 reuse per column half). Splitting by column half
        # lets the g=1 column-A reload start while the reserved g=0
        # column-B v-projections are still draining behind the c=0 pass.
        xh = {}

        def dma_x_ch(g, hf):
            for pl, xp in ((4, x4p), (5, x5p)):
                xh[(g, pl, hf)] = x_p.tile([P, NDT, 512],
                                           F8H if pl == 4 else F8L,
                                           tag=f"x{pl}h{hf}",
                                           name=f"x{pl}h{g}_{hf}")
            for tq in range(4):
                for pl, xp in ((4, x4p), (5, x5p)):
                    nc.sync.dma_start(
                        out=xh[(g, pl, hf)][:, 4 * tq:4 * tq + 4, :],
                        in_=xp[:, 4 * tq:4 * tq + 4,
                               g * 1024 + hf * 512:
                               g * 1024 + (hf + 1) * 512])

        # ---------- projection unit builders ----------
        def qk_head_units(units, g, h, w4p, w5p, dst, nm):
            """q or k projection for head h, seq half g -> dst[:,h,g*1024:]."""
            cell = {}

            def dma(cell=cell):
                cell["w4"] = wqk_p.tile([P, NDT, P], F8H, tag="w4",
                                        name=f"w4{nm}{g}_{h}")
                cell["w5"] = wqk_p.tile([P, NDT, P], F8L, tag="w5",
                                        name=f"w5{nm}{g}_{h}")
                nc.sync.dma_start(out=cell["w4"][:], in_=w4p[:, h, :, :])
                nc.sync.dma_start(out=cell["w5"][:], in_=w5p[:, h, :, :])
            units.add(0, dma)
            for hf in range(2):
                for t in range(NDP):
                    for mi in range(3):
                        def mm(t=t, mi=mi, hf=hf, cell=cell):
                            if t == 0 and mi == 0:
                                cell["pq"] = pp_p.tile(
                                    [P, 512], F32, tag="pp",
                                    name=f"p{nm}{g}_{h}_{hf}")
                            sl = slice(2 * t, 2 * t + 2)
                            s = cell["w4" if mi < 2 else "w5"][:, sl, :]
                            m = xh[(g, 5 if mi == 1 else 4, hf)][:, sl, :]
                            nc.tensor.matmul(
                                cell["pq"][:], s, m,
                                start=(t == 0 and mi == 0),
                                stop=(t == NDP - 1 and mi == 2),
                                perf_mode=DRM)
                        units.add(128, mm)

                def evac(hf=hf, cell=cell):
                    # split evacuations between Act (q) and DVE (k) so
                    # neither engine saturates during overlap windows
                    d = dst[:, h, g * 1024 + hf * 512:
                            g * 1024 + (hf + 1) * 512]
                    if nm == "q":
                        nc.scalar.copy(d, cell["pq"][:])
                    else:
                        nc.vector.tensor_copy(d, cell["pq"][:])
                units.add(0, evac)

        def v_dma_unit(units, g, mh, cell):
            def dma(cell=cell, mh=mh, g=g):
                cell["wv4"] = wv_p.tile([P, NDT, 512], F8H, tag="wv4",
                                        name=f"wv4{g}_{mh}")
                cell["wv5"] = wv_p.tile([P, NDT, 512], F8L, tag="wv5",
                                        name=f"wv5{g}_{mh}")
                nc.sync.dma_start(
                    out=cell["wv4"][:],
                    in_=wv4p[:, :, mh * 512:(mh + 1) * 512])
                nc.sync.dma_start(
                    out=cell["wv5"][:],
                    in_=wv5p[:, :, mh * 512:(mh + 1) * 512])
            units.add(0, dma)

        def v_group_units(units, g, mh, cell, sts=range(8)):
            for st in sts:
                stg = 8 * g + st
                for t in range(NDP):
                    for mi in range(3):
                        def mm(t=t, mi=mi, stg=stg, mh=mh, cell=cell, g=g):
                            if t == 0 and mi == 0:
                                cell["pv"] = pp_p.tile(
                                    [P, 512], F32, tag="pp",
                                    name=f"pv{stg}_{mh}")
                            sl = slice(2 * t, 2 * t + 2)
                            stl = stg - 8 * g
                            so = (stl % 4) * P
                            s = xh[(g, 5 if mi == 2 else 4, stl // 4)][
                                :, sl, so:so + P]
                            m = cell["wv5" if mi == 1 else "wv4"][:, sl, :]
                            nc.tensor.matmul(
                                cell["pv"][:], s, m,
                                start=(t == 0 and mi == 0),
                                stop=(t == NDP - 1 and mi == 2),
                                perf_mode=DRM)
                        units.add(128, mm)
                units.add(0, lambda stg=stg, mh=mh, cell=cell:
                          nc.vector.tensor_copy(
                              vres[:, stg, mh * 512:(mh + 1) * 512],
                              cell["pv"][:]))

        def proj_phase_units(g, reserve=None):
            # head-0 q/k first (small weight DMAs -> earliest PE start);
            # each v m-half's weight DMA is sandwiched between qk heads so
            # its write-after-read wait on the previous half's buffer
            # overlaps projection compute instead of stalling the PE.
            # With `reserve`, the m-half-1 v groups for s-tiles 4..7 go to
            # the reserve queue (PE filler for the following c=0 pass).
            units = Units()
            cell0, cell1 = {}, {}
            qk_head_units(units, g, 0, wk4p, wk5p, kres, "k")
            qk_head_units(units, g, 0, wq4p, wq5p, qres, "q")
            v_dma_unit(units, g, 0, cell0)
            v_group_units(units, g, 0, cell0)
            qk_head_units(units, g, 1, wk4p, wk5p, kres, "k")
            qk_head_units(units, g, 1, wq4p, wq5p, qres, "q")
            v_dma_unit(units, g, 1, cell1)
            if reserve is None:
                v_group_units(units, g, 1, cell1)
            else:
                v_group_units(units, g, 1, cell1, sts=range(0, 4))
                v_group_units(reserve, g, 1, cell1, sts=range(4, 8))
            for h in range(2, HL):
                qk_head_units(units, g, h, wk4p, wk5p, kres, "k")
                qk_head_units(units, g, h, wq4p, wq5p, qres, "q")
            return units

        # ---------- out-projection ----------
        wo4s = [None]
        wo5s = [None]

        def oproj_units(units, st_list, use_act):
            for st in st_list:
                for q4 in range(4):
                    po = [None]
                    for hp in range(4):
                        for mi in range(3):
                            def mm(hp=hp, mi=mi, st=st, q4=q4, po=po):
                                if hp == 0 and mi == 0:
                                    po[0] = pp_p.tile([P, 512], F32,
                                                      tag="pp",
                                                      name=f"po{st}_{q4}")
                                sl = slice(2 * hp, 2 * hp + 2)
                                ss = st * P
                                qs = q4 * 512
                                s = (ctx4 if mi < 2 else ctx5)[
                                    :, sl, ss:ss + P]
                                m = (wo5s if mi == 1 else wo4s)[0][
                                    :, sl, qs:qs + 512]
                                nc.tensor.matmul(
                                    po[0][:], s, m,
                                    start=(hp == 0 and mi == 0),
                                    stop=(hp == 3 and mi == 2),
                                    perf_mode=DRM)
                            units.add(128, mm)
                    def evac(st=st, q4=q4, po=po):
                        ev = ostg_p.tile([P, 512], F16, tag="ostg",
                                         name=f"ostg{st}_{q4}")
                        if use_act and q4 % 2 == 0:
                            nc.scalar.copy(ev[:], po[0][:])
                        else:
                            nc.vector.tensor_copy(ev[:], po[0][:])
                        nc.sync.dma_start(
                            out=outp[st * P:(st + 1) * P,
                                     q4 * 512:(q4 + 1) * 512],
                            in_=ev[:])
                    units.add(0, evac)

        # ---------- attention ----------
        def attn_chunk(h, c, bg, scale=1.0, finish_prev=None):
            nj, ndiag = 4 * c + 4, 4 * c
            d1, d2, d3 = int(250 * scale), int(200 * scale), int(400 * scale)
            acc = acc_p.tile([P, 512], F16, tag="acc", name=f"acc{h}_{c}")
            pctx = pctx_p.tile([P, 512], F32, tag="pctx", name=f"px{h}_{c}")
            pend = deque()

            def emit_av(last):
                Jp, rp, ptp = pend.popleft()
                nc.tensor.matmul(
                    pctx[:, rp:512],
                    vres[:, Jp, h * P:(h + 1) * P], ptp[:, rp:512],
                    start=(Jp == 0), stop=last)

            for J in range(nj):
                r = J * P - c * 512 if J >= ndiag else 0
                pscr = pscr_p.tile([P, 512], F32, tag="pscr",
                                   name=f"ps{h}_{c}_{J}")
                nc.tensor.matmul(
                    pscr[:, r:512],
                    kres[:, h, J * P:(J + 1) * P],
                    qres[:, h, c * 512 + r:(c + 1) * 512],
                    start=True, stop=True)
                bg.drain(d1)
                pt = pt_p.tile([P, 512], F16, tag="pt",
                               name=f"pt{h}_{c}_{J}")
                nc.scalar.activation(pt[:, r:512], pscr[:, r:512], Exp,
                                     bias=ebias[:], scale=ESCALE)
                if J >= ndiag:
                    nc.vector.tensor_mul(pt[:, r:r + P], pt[:, r:r + P],
                                         m0[:])
                if J == 0:
                    if c < 2:
                        nc.scalar.copy(acc[:], pt[:])
                    else:
                        nc.vector.tensor_copy(acc[:], pt[:])
                else:
                    nc.vector.tensor_add(acc[:, r:512], acc[:, r:512],
                                         pt[:, r:512])
                pend.append((J, r, pt))
                # lag-2: attn@V for J-2 — two full J periods behind its
                # exp(), so the Act latency never stalls the PE
                if len(pend) > 2:
                    emit_av(False)
                    bg.drain(d2)
                if J == 1 and finish_prev is not None:
                    finish_prev()
                    finish_prev = None
            if finish_prev is not None:
                finish_prev()
            while pend:
                emit_av(len(pend) == 1)
            # softmax denominator: reduce+broadcast on the idle GPSIMD engine
            den = nrm_p.tile([P, 512], F32, tag="den", name=f"den{h}_{c}")
            nc.gpsimd.partition_all_reduce(
                den[:], acc[:], channels=P, reduce_op=bass_isa.ReduceOp.add)

            def finish(bg2=bg, d3=d3, fast=(c == 3 and h == HL - 1)):
                # deferred tail of the normalization: emitted early in the
                # NEXT chunk so the DVE never head-of-line blocks on the
                # Pool all_reduce. GPSIMD cannot read PSUM, so the pctx
                # multiply stays on DVE; the SBUF-only ops go to Pool.
                rec = rec_p.tile([P, 512], F32, tag="rec",
                                 name=f"rec{h}_{c}")
                nc.vector.reciprocal(rec[:], den[:])
                bg2.drain(d3)
                cv = nrm_p.tile([P, 512], F16, tag="cv", name=f"cv{h}_{c}")
                nc.vector.tensor_mul(cv[:], pctx[:], rec[:])
                cs = slice(c * 512, (c + 1) * 512)
                if fast:
                    nc.scalar.copy(ctx4[:, h, cs], cv[:])
                    nc.vector.tensor_sub(ctx5[:, h, cs], cv[:],
                                         ctx4[:, h, cs])
                else:
                    nc.gpsimd.tensor_copy(ctx4[:, h, cs], cv[:])
                    nc.gpsimd.tensor_sub(ctx5[:, h, cs], cv[:],
                                         ctx4[:, h, cs])
                bg2.drain(d3)
            return finish

        # ================= schedule =================
        # Scarce background supply is rate-matched to the Act-bound
        # attention rate (~144 ns of filler per key tile): PE stays filled
        # exactly while exp() limits, leftovers run as pure PE stretches.
        RM = 0.75
        empty = Units()
        g0t = Units()
        g0 = proj_phase_units(0, reserve=g0t)
        g0.drain(1)     # head-0 weight DMAs issue before the x bulk load
        dma_x_ch(0, 0)
        dma_x_ch(0, 1)
        nc.sync.dma_start(out=m0[:], in_=mask0[:])
        g0.drain_all()
        # column A of the half-1 x reload: its WAR readers (all column-A
        # g=0 projections) are fully emitted, so it streams during c=0
        dma_x_ch(1, 0)

        g1 = proj_phase_units(1)
        fin = None
        for h in range(HL):
            # c=0 chunks are short (~3us): spread the whole v reserve across
            # them
            fin = attn_chunk(h, 0, g0t, scale=1.2, finish_prev=fin)
        g0t.drain_all()
        # column B follows once the reserved column-B readers are done
        dma_x_ch(1, 1)
        for h in range(HL):
            fin = attn_chunk(h, 1, g1, scale=RM, finish_prev=fin)
        fin()
        fin = None
        g1.drain_all()
        ph1.close()

        # phase 3 SBUF (reuses the space freed by x/weight pools)
        ph3 = ExitStack()
        wo_p = ph3.enter_context(tc.tile_pool(name="wop", bufs=1,
                                              side="right"))
        wo4s[0] = wo_p.tile([P, HL, D], F8H, tag="wo4s", name="wo4s")
        wo5s[0] = wo_p.tile([P, HL, D], F8L, tag="wo5s", name="wo5s")
        # per-head-pair loads: the first out-proj triple only needs the
        # hp=0 rows, so it can start ~1.5us after phase entry
        for hp in range(4):
            hs = slice(2 * hp, 2 * hp + 2)
            nc.sync.dma_start(out=wo4s[0][:, hs, :], in_=wo4p[:, hs, :])
            nc.sync.dma_start(out=wo5s[0][:, hs, :], in_=wo5p[:, hs, :])

        b2 = Units()
        oproj_units(b2, range(0, 8), use_act=True)
        for h in range(HL):
            # h<2: no drains, so the ~12us wo weight DMA completes behind
            # attention instead of stalling the first out-proj triple
            fin = attn_chunk(h, 2, b2 if h >= 1 else empty, scale=RM,
                             finish_prev=fin)
        oproj_units(b2, range(8, 12), use_act=True)
        for h in range(HL):
            fin = attn_chunk(h, 3, b2, scale=0.7, finish_prev=fin)
        fin()
        oproj_units(b2, range(12, 16), use_act=False)
        b2.drain_all()
        ph3.close()

    nc.finalize()
    return nc


def get_nc():
    if "nc" not in _BUILT:
        _BUILT["nc"] = _build()
    return _BUILT["nc"]


def _f8split(a):
    """Split fp32 into an e4m3 hi plane and an e5m2 lo residual plane."""
    a = np.ascontiguousarray(a, dtype=np.float32)
    hi = a.astype(ml_dtypes.float8_e4m3)
    lo = (a - hi.astype(np.float32)).astype(ml_dtypes.float8_e5m2)
    return hi, lo


def _make_in_maps(x, Wq, Wk, Wv, Wo):
    jj, ff = np.meshgrid(np.arange(P), np.arange(P), indexing="ij")
    m0 = (ff >= jj).astype(np.float16)
    in_maps = []
    for c in range(8):
        b, t = c // 2, c % 2
        ms = slice(t * ML, (t + 1) * ML)
        # x^T [D,S] -> [P, NDT, S]
        x4, x5 = _f8split(
            x[b].T.reshape(NDT, P, S).transpose(1, 0, 2))
        # Wq/Wk slices: (SW*W[ms,:]).T [D, ML] -> [P, HL, NDT, P]
        wq4, wq5 = _f8split(
            (SW * Wq[ms, :]).T.reshape(NDT, P, HL, P).transpose(1, 2, 0, 3))
        wk4, wk5 = _f8split(
            (SW * Wk[ms, :]).T.reshape(NDT, P, HL, P).transpose(1, 2, 0, 3))
        # Wv: (SW*Wv[ms,:]).T [D, ML] -> [P, NDT, ML]
        wv4, wv5 = _f8split(
            (SW * Wv[ms, :]).T.reshape(NDT, P, ML).transpose(1, 0, 2))
        # Wo: (SW*Wo[:,ms]).T [ML, D] -> [P, HL, D]
        wo4, wo5 = _f8split(
            (SW * Wo[:, ms]).T.reshape(HL, P, D).transpose(1, 0, 2))
        in_maps.append({
            "x4": x4, "x5": x5,
            "wq4": wq4, "wq5": wq5,
            "wk4": wk4, "wk5": wk5,
            "wv4": wv4, "wv5": wv5,
            "wo4": wo4, "wo5": wo5,
            "mask0": m0,
        })
    return in_maps


def kernel(x, Wq, Wk, Wv, Wo):
    x = np.asarray(x, dtype=np.float32)
    Wq = np.asarray(Wq, dtype=np.float32)
    Wk = np.asarray(Wk, dtype=np.float32)
    Wv = np.asarray(Wv, dtype=np.float32)
    Wo = np.asarray(Wo, dtype=np.float32)

    nc = get_nc()
    in_maps = _make_in_maps(x, Wq, Wk, Wv, Wo)
    res = run_bass_kernel_spmd(nc, in_maps, list(range(8)))
    outs = [res.results[c]["out"].astype(np.float32) for c in range(8)]
    full = np.stack([(outs[2 * b] + outs[2 * b + 1]) * OSCALE
                     for b in range(B)])
    return full.astype(np.float32)
